# revision 7
# baseline (speedup 1.0000x reference)
"""Trainium2 Bass kernel for nn_CNNEmbedding (embedding lookup -> BN -> GELU -> matmul -> BN -> GELU).

Strategy (8 NeuronCores, data-parallel over the token dim):
  - Host: W1 is transposed to an embedding-table layout [vocab, inter] in bf16 so
    each token lookup is one contiguous row; W2 / g / beta are pre-laid-out per
    partition; x_t is split into 8 shards of 1024 tokens.
  - Device (per core): dma_gather(transpose=True) pulls the 1024 gathered rows
    directly into channel-major SBUF tiles [128, 34, tokens]; BatchNorm stats
    (sum / sum-of-squares per channel) are computed locally and all-reduced
    across the 8 cores (b1 cancels inside BatchNorm so it is skipped);
    normalize+GELU fuse into one ScalarE activation per channel chunk; the
    [tokens, 4352] @ [4352, 512] matmul accumulates in PSUM over 34 chunk
    matmuls; BN2 stats are all-reduced (b2 cancels too) and the final
    normalize+GELU writes the output emb-major, which the host transposes.
"""

import numpy as np
import ml_dtypes

BF16 = ml_dtypes.bfloat16

VOCAB = 8192
INTER = 4352
EMB = 512
N = 8192
NCORES = 8
NT = N // NCORES          # tokens per core
C = INTER // 128          # 34 channel chunks
CE = EMB // 128           # 4 emb chunks
PIECES = 2
PT = NT // PIECES         # 512 tokens per gather piece
EPS = 1e-5

_CACHE = {}


def _build_program():
    if "nc" in _CACHE:
        return _CACHE["nc"]

    import concourse.bacc as bacc
    from concourse import mybir, tile

    f32 = mybir.dt.float32
    bf16 = mybir.dt.bfloat16
    i16 = mybir.dt.int16
    AF = mybir.ActivationFunctionType
    ALU = mybir.AluOpType
    AX = mybir.AxisListType

    nc = bacc.Bacc("TRN2", target_bir_lowering=False, debug=False, num_devices=NCORES,
                   num_swdge_queues=PIECES, dynamic_dma_scratch_size=32768)

    table = nc.dram_tensor("table", [VOCAB, INTER], bf16, kind="ExternalInput")
    idx = nc.dram_tensor("idx", [128, NT // 16], i16, kind="ExternalInput")
    w2t = nc.dram_tensor("w2t", [128, C, EMB], bf16, kind="ExternalInput")
    gb1 = nc.dram_tensor("gb1", [128, 2, C], f32, kind="ExternalInput")
    gb2 = nc.dram_tensor("gb2", [128, 2, CE], f32, kind="ExternalInput")
    out = nc.dram_tensor("out", [128, CE, NT], f32, kind="ExternalOutput")

    RG = [list(range(NCORES))]

    with tile.TileContext(nc) as tc:
        with (
            tc.tile_pool(name="sb", bufs=1) as sb,
            tc.tile_pool(name="ps", bufs=1, space="PSUM") as ps,
            tc.tile_pool(name="dram", bufs=1, space="DRAM") as dram,
        ):
            idx_sb = sb.tile([128, NT // 16], i16, tag="idx", name="idx")
            nc.scalar.dma_start(idx_sb[:], idx[:])
            w2t_sb = sb.tile([128, C, EMB], bf16, tag="w2t", name="w2t")
            nc.sync.dma_start(w2t_sb[:], w2t[:])
            gb1_sb = sb.tile([128, 2, C], f32, tag="gb1", name="gb1")
            nc.scalar.dma_start(gb1_sb[:], gb1[:])
            gb2_sb = sb.tile([128, 2, CE], f32, tag="gb2", name="gb2")
            nc.scalar.dma_start(gb2_sb[:], gb2[:])

            # ---- phase 1: gather the embedding rows channel-major ----
            h1 = []
            for q in range(PIECES):
                g = sb.tile([128, C, PT], bf16, tag=f"h1_{q}", name=f"h1_{q}")
                nc.gpsimd.dma_gather(
                    out_ap=g[:],
                    in_ap=table[:],
                    idxs_ap=idx_sb[:, q * (PT // 16):(q + 1) * (PT // 16)],
                    num_idxs=PT,
                    num_idxs_reg=PT,
                    elem_size=INTER,
                    transpose=True,
                    queue_num=q,
                )
                h1.append(g)

            # ---- local BN1 stats: per-channel sum (DVE) and sumsq (ACT) ----
            arin1 = sb.tile([128, 2, C], f32, tag="arin1", name="arin1")
            sums = [sb.tile([128, C], f32, tag=f"sum{q}", name=f"sum{q}") for q in range(PIECES)]
            ssqs = [sb.tile([128, C], f32, tag=f"ssq{q}", name=f"ssq{q}") for q in range(PIECES)]
            trash = sb.tile([128, PT], bf16, tag="trash", name="trash")
            for q in range(PIECES):
                nc.vector.reduce_sum(sums[q][:], h1[q][:], axis=AX.X)
                for c in range(C):
                    nc.scalar.activation(
                        trash[:], h1[q][:, c, :], AF.Square,
                        accum_out=ssqs[q][:, c:c + 1],
                    )
            nc.vector.tensor_add(arin1[:, 0, :], sums[0][:], sums[1][:])
            nc.vector.tensor_add(arin1[:, 1, :], ssqs[0][:], ssqs[1][:])

            # ---- all-reduce BN1 stats across the 8 cores ----
            ar1_in = dram.tile([128, 2, C], f32, tag="ar1i", name="ar1i")
            ar1_out = dram.tile([128, 2, C], f32, tag="ar1o", name="ar1o")
            nc.sync.dma_start(ar1_in[:], arin1[:])
            nc.gpsimd.collective_compute(
                "AllReduce", ALU.add, replica_groups=RG,
                ins=[ar1_in.opt()], outs=[ar1_out.opt()],
            )
            arout1 = sb.tile([128, 2, C], f32, tag="arout1", name="arout1")
            nc.sync.dma_start(arout1[:], ar1_out[:])

            # ---- scale/shift: scale = g1*rsqrt(var+eps), shift = beta1-scale*mean
            mean1 = sb.tile([128, C], f32, tag="mean1", name="mean1")
            var1 = sb.tile([128, C], f32, tag="var1", name="var1")
            mm1 = sb.tile([128, C], f32, tag="mm1", name="mm1")
            rstd1 = sb.tile([128, C], f32, tag="rstd1", name="rstd1")
            scale1 = sb.tile([128, C], f32, tag="scale1", name="scale1")
            shift1 = sb.tile([128, C], f32, tag="shift1", name="shift1")
            nc.vector.tensor_scalar_mul(mean1[:], arout1[:, 0, :], 1.0 / N)
            nc.vector.tensor_mul(mm1[:], mean1[:], mean1[:])
            nc.vector.scalar_tensor_tensor(
                var1[:], arout1[:, 1, :], 1.0 / N, mm1[:],
                op0=ALU.mult, op1=ALU.subtract,
            )
            nc.vector.tensor_scalar_add(var1[:], var1[:], EPS)
            nc.scalar.activation(rstd1[:], var1[:], AF.Sqrt)
            nc.vector.reciprocal(rstd1[:], rstd1[:])
            # one Newton step for rsqrt accuracy: r = r*(1.5 - 0.5*v*r^2)
            nc.vector.tensor_mul(mm1[:], rstd1[:], rstd1[:])
            nc.vector.tensor_mul(mm1[:], mm1[:], var1[:])
            nc.vector.tensor_scalar(
                mm1[:], mm1[:], -0.5, 1.5, op0=ALU.mult, op1=ALU.add,
            )
            nc.vector.tensor_mul(rstd1[:], rstd1[:], mm1[:])
            nc.vector.tensor_mul(scale1[:], gb1_sb[:, 0, :], rstd1[:])
            nc.vector.tensor_mul(mm1[:], scale1[:], mean1[:])
            nc.vector.tensor_sub(shift1[:], gb1_sb[:, 1, :], mm1[:])

            # ---- normalize + GELU (in place, bf16), then matmul into PSUM ----
            for q in range(PIECES):
                scale_b = scale1[:].to_broadcast([128, C, PT])
                shift_b = shift1[:].to_broadcast([128, C, PT])
                nc.vector.tensor_mul(h1[q][:], h1[q][:], scale_b)
                nc.vector.tensor_add(h1[q][:], h1[q][:], shift_b)
                nc.scalar.activation(h1[q][:], h1[q][:], AF.Gelu)

            ps_out = [ps.tile([128, NT], f32, tag=f"po{e}", name=f"po{e}") for e in range(CE)]
            for c in range(C):
                for e in range(CE):
                    lhsT = w2t_sb[:, c, e * 128:(e + 1) * 128]
                    for q in range(PIECES):
                        nc.tensor.matmul(
                            ps_out[e][:, q * PT:(q + 1) * PT],
                            lhsT,
                            h1[q][:, c, :],
                            start=(c == 0),
                            stop=(c == C - 1),
                        )

            # ---- BN2 stats (local) + all-reduce ----
            arin2 = sb.tile([128, 2, CE], f32, tag="arin2", name="arin2")
            trash2 = sb.tile([128, NT], bf16, tag="trash2", name="trash2")
            for e in range(CE):
                nc.vector.reduce_sum(arin2[:, 0, e:e + 1], ps_out[e][:], axis=AX.X)
                nc.scalar.activation(
                    trash2[:], ps_out[e][:], AF.Square,
                    accum_out=arin2[:, 1, e:e + 1],
                )
            ar2_in = dram.tile([128, 2, CE], f32, tag="ar2i", name="ar2i")
            ar2_out = dram.tile([128, 2, CE], f32, tag="ar2o", name="ar2o")
            nc.sync.dma_start(ar2_in[:], arin2[:])
            nc.gpsimd.collective_compute(
                "AllReduce", ALU.add, replica_groups=RG,
                ins=[ar2_in.opt()], outs=[ar2_out.opt()],
            )
            arout2 = sb.tile([128, 2, CE], f32, tag="arout2", name="arout2")
            nc.sync.dma_start(arout2[:], ar2_out[:])

            mean2 = sb.tile([128, CE], f32, tag="mean2", name="mean2")
            var2 = sb.tile([128, CE], f32, tag="var2", name="var2")
            mm2 = sb.tile([128, CE], f32, tag="mm2", name="mm2")
            rstd2 = sb.tile([128, CE], f32, tag="rstd2", name="rstd2")
            scale2 = sb.tile([128, CE], f32, tag="scale2", name="scale2")
            shift2 = sb.tile([128, CE], f32, tag="shift2", name="shift2")
            nc.vector.tensor_scalar_mul(mean2[:], arout2[:, 0, :], 1.0 / N)
            nc.vector.tensor_mul(mm2[:], mean2[:], mean2[:])
            nc.vector.scalar_tensor_tensor(
                var2[:], arout2[:, 1, :], 1.0 / N, mm2[:],
                op0=ALU.mult, op1=ALU.subtract,
            )
            nc.vector.tensor_scalar_add(var2[:], var2[:], EPS)
            nc.scalar.activation(rstd2[:], var2[:], AF.Sqrt)
            nc.vector.reciprocal(rstd2[:], rstd2[:])
            nc.vector.tensor_mul(mm2[:], rstd2[:], rstd2[:])
            nc.vector.tensor_mul(mm2[:], mm2[:], var2[:])
            nc.vector.tensor_scalar(
                mm2[:], mm2[:], -0.5, 1.5, op0=ALU.mult, op1=ALU.add,
            )
            nc.vector.tensor_mul(rstd2[:], rstd2[:], mm2[:])
            nc.vector.tensor_mul(scale2[:], gb2_sb[:, 0, :], rstd2[:])
            nc.vector.tensor_mul(mm2[:], scale2[:], mean2[:])
            nc.vector.tensor_sub(shift2[:], gb2_sb[:, 1, :], mm2[:])

            # ---- final normalize + GELU, write emb-major output ----
            out_sb = sb.tile([128, CE, NT], f32, tag="out", name="out")
            for e in range(CE):
                nc.scalar.activation(
                    out_sb[:, e, :], ps_out[e][:], AF.Gelu,
                    bias=shift2[:, e:e + 1], scale=scale2[:, e:e + 1],
                )
            nc.sync.dma_start(out[:], out_sb[:])

    nc.compile()
    _CACHE["nc"] = nc
    return nc


def kernel(x_t, W1, b1, g1, beta1, W2, b2, g2, beta2):
    from concourse.bass_utils import run_bass_kernel_spmd

    nc = _build_program()

    # b1/b2 cancel inside the BatchNorms (mean subtraction), so they are unused.
    table = np.ascontiguousarray(np.asarray(W1, dtype=np.float32).T).astype(BF16)
    w2t = np.ascontiguousarray(
        np.asarray(W2, dtype=np.float32).reshape(EMB, C, 128).transpose(2, 1, 0)
    ).astype(BF16)
    g1r = np.asarray(g1, dtype=np.float32).reshape(C, 128).T
    b1r = np.asarray(beta1, dtype=np.float32).reshape(C, 128).T
    gb1 = np.ascontiguousarray(np.stack([g1r, b1r], axis=1))      # [128, 2, C]
    g2r = np.asarray(g2, dtype=np.float32).reshape(CE, 128).T
    b2r = np.asarray(beta2, dtype=np.float32).reshape(CE, 128).T
    gb2 = np.ascontiguousarray(np.stack([g2r, b2r], axis=1))      # [128, 2, CE]

    x = np.asarray(x_t).astype(np.int64)
    in_maps = []
    for i in range(NCORES):
        xl = x[i * NT:(i + 1) * NT].astype(np.int16)
        wrapped = xl.reshape(NT // 16, 16).T                      # [16, NT//16]
        idx = np.ascontiguousarray(np.tile(wrapped, (8, 1)))      # [128, NT//16]
        in_maps.append(
            {"table": table, "idx": idx, "w2t": w2t, "gb1": gb1, "gb2": gb2}
        )

    _CACHE["in_maps"] = in_maps
    res = run_bass_kernel_spmd(nc, in_maps, list(range(NCORES)))

    shards = []
    for i in range(NCORES):
        o = res.results[i]["out"]                                 # [128, CE, NT]
        shards.append(o.transpose(2, 1, 0).reshape(NT, EMB))      # [NT, EMB]
    return np.ascontiguousarray(np.concatenate(shards, axis=0)).astype(np.float32)


# revision 9
# speedup vs baseline: 1.4321x; 1.4321x over previous
"""Trainium2 Bass kernel for nn_CNNEmbedding (embedding lookup -> BN -> GELU -> matmul -> BN -> GELU).

Strategy (8 NeuronCores, data-parallel over the token dim):
  - Host: W1 is transposed to an embedding-table layout [vocab, inter] in bf16 so
    each token lookup is one contiguous row; W2 / g / beta are pre-laid-out per
    partition; x_t is split into 8 shards of 1024 tokens.
  - Device (per core): dma_gather(transpose=True) pulls the 1024 gathered rows
    directly into channel-major SBUF tiles [128, 34, tokens]; BatchNorm stats
    (sum / sum-of-squares per channel) are computed locally and all-reduced
    across the 8 cores (b1 cancels inside BatchNorm so it is skipped);
    normalize+GELU fuse into one ScalarE activation per channel chunk; the
    [tokens, 4352] @ [4352, 512] matmul accumulates in PSUM over 34 chunk
    matmuls; BN2 stats are all-reduced (b2 cancels too) and the final
    normalize+GELU writes the output emb-major, which the host transposes.
"""

import numpy as np
import ml_dtypes

BF16 = ml_dtypes.bfloat16

VOCAB = 8192
INTER = 4352
EMB = 512
N = 8192
NCORES = 8
NT = N // NCORES          # tokens per core
C = INTER // 128          # 34 channel chunks
CE = EMB // 128           # 4 emb chunks
PIECES = 2
PT = NT // PIECES         # 512 tokens per gather piece
EPS = 1e-5

_CACHE = {}


def _build_program():
    if "nc" in _CACHE:
        return _CACHE["nc"]

    import concourse.bacc as bacc
    from concourse import mybir, tile

    f32 = mybir.dt.float32
    bf16 = mybir.dt.bfloat16
    i16 = mybir.dt.int16
    AF = mybir.ActivationFunctionType
    ALU = mybir.AluOpType
    AX = mybir.AxisListType

    nc = bacc.Bacc("TRN2", target_bir_lowering=False, debug=False, num_devices=NCORES,
                   num_swdge_queues=PIECES, dynamic_dma_scratch_size=32768)

    table = nc.dram_tensor("table", [VOCAB, INTER], bf16, kind="ExternalInput")
    idx = nc.dram_tensor("idx", [128, NT // 16], i16, kind="ExternalInput")
    w2t = nc.dram_tensor("w2t", [128, C, EMB], bf16, kind="ExternalInput")
    gb1 = nc.dram_tensor("gb1", [128, 2, C], f32, kind="ExternalInput")
    gb2 = nc.dram_tensor("gb2", [128, 2, CE], f32, kind="ExternalInput")
    out = nc.dram_tensor("out", [128, CE, NT], f32, kind="ExternalOutput")

    RG = [list(range(NCORES))]

    with tile.TileContext(nc) as tc:
        with (
            tc.tile_pool(name="sb", bufs=1) as sb,
            tc.tile_pool(name="ps", bufs=1, space="PSUM") as ps,
            tc.tile_pool(name="dram", bufs=1, space="DRAM") as dram,
        ):
            idx_sb = sb.tile([128, NT // 16], i16, tag="idx", name="idx")
            nc.scalar.dma_start(idx_sb[:], idx[:])
            w2t_sb = sb.tile([128, C, EMB], bf16, tag="w2t", name="w2t")
            nc.sync.dma_start(w2t_sb[:], w2t[:])
            gb1_sb = sb.tile([128, 2, C], f32, tag="gb1", name="gb1")
            nc.scalar.dma_start(gb1_sb[:], gb1[:])
            gb2_sb = sb.tile([128, 2, CE], f32, tag="gb2", name="gb2")
            nc.scalar.dma_start(gb2_sb[:], gb2[:])

            # ---- phase 1: gather the embedding rows channel-major ----
            h1 = []
            for q in range(PIECES):
                g = sb.tile([128, C, PT], bf16, tag=f"h1_{q}", name=f"h1_{q}")
                nc.gpsimd.dma_gather(
                    out_ap=g[:],
                    in_ap=table[:],
                    idxs_ap=idx_sb[:, q * (PT // 16):(q + 1) * (PT // 16)],
                    num_idxs=PT,
                    num_idxs_reg=PT,
                    elem_size=INTER,
                    transpose=True,
                    queue_num=q,
                )
                h1.append(g)

            # ---- local BN1 stats: per-channel sum and sumsq ----
            # chunks [0, CD) -> DVE bn_stats (mean/var in one pass);
            # chunks [CD, C) -> ACT Identity+accum (sum) and Square+accum (sumsq)
            CD = 24
            arin1 = sb.tile([128, 2, C], f32, tag="arin1", name="arin1")
            sums = [sb.tile([128, C], f32, tag=f"sum{q}", name=f"sum{q}") for q in range(PIECES)]
            ssqs = [sb.tile([128, C], f32, tag=f"ssq{q}", name=f"ssq{q}") for q in range(PIECES)]
            st6 = [sb.tile([128, CD, 6], f32, tag=f"st6_{q}", name=f"st6_{q}") for q in range(PIECES)]
            trash = sb.tile([128, PT], bf16, tag="trash", name="trash")
            cvt = [sb.tile([128, 3, CD], f32, tag=f"cvt{q}", name=f"cvt{q}") for q in range(PIECES)]
            for q in range(PIECES):
                for c in range(CD):
                    nc.vector.bn_stats(st6[q][:, c, :], h1[q][:, c, :])
                for c in range(CD, C):
                    nc.scalar.activation(
                        trash[:], h1[q][:, c, :], AF.Identity,
                        accum_out=sums[q][:, c:c + 1],
                    )
                    nc.scalar.activation(
                        trash[:], h1[q][:, c, :], AF.Square,
                        accum_out=ssqs[q][:, c:c + 1],
                    )
                # convert (count, mean, M2) x {even, odd} -> sum, sumsq
                t0 = st6[q][:, :, 0]; t1 = st6[q][:, :, 1]; t2 = st6[q][:, :, 2]
                t3 = st6[q][:, :, 3]; t4 = st6[q][:, :, 4]; t5 = st6[q][:, :, 5]
                m01 = cvt[q][:, 0, :]; m34 = cvt[q][:, 1, :]; tmp = cvt[q][:, 2, :]
                nc.vector.tensor_mul(m01, t0, t1)
                nc.vector.tensor_mul(m34, t3, t4)
                nc.vector.tensor_add(sums[q][:, 0:CD], m01, m34)
                nc.vector.tensor_add(tmp, t2, t5)
                nc.vector.tensor_mul(m01, m01, t1)
                nc.vector.tensor_mul(m34, m34, t4)
                nc.vector.tensor_add(tmp, tmp, m01)
                nc.vector.tensor_add(ssqs[q][:, 0:CD], tmp, m34)
            nc.vector.tensor_add(arin1[:, 0, :], sums[0][:], sums[1][:])
            nc.vector.tensor_add(arin1[:, 1, :], ssqs[0][:], ssqs[1][:])

            # ---- all-reduce BN1 stats across the 8 cores ----
            ar1_in = dram.tile([128, 2, C], f32, tag="ar1i", name="ar1i")
            ar1_out = dram.tile([128, 2, C], f32, tag="ar1o", name="ar1o")
            nc.sync.dma_start(ar1_in[:], arin1[:])
            nc.gpsimd.collective_compute(
                "AllReduce", ALU.add, replica_groups=RG,
                ins=[ar1_in.opt()], outs=[ar1_out.opt()],
            )
            arout1 = sb.tile([128, 2, C], f32, tag="arout1", name="arout1")
            nc.sync.dma_start(arout1[:], ar1_out[:])

            # ---- scale/shift: scale = g1*rsqrt(var+eps), shift = beta1-scale*mean
            mean1 = sb.tile([128, C], f32, tag="mean1", name="mean1")
            var1 = sb.tile([128, C], f32, tag="var1", name="var1")
            mm1 = sb.tile([128, C], f32, tag="mm1", name="mm1")
            rstd1 = sb.tile([128, C], f32, tag="rstd1", name="rstd1")
            scale1 = sb.tile([128, C], f32, tag="scale1", name="scale1")
            shift1 = sb.tile([128, C], f32, tag="shift1", name="shift1")
            nc.vector.tensor_scalar_mul(mean1[:], arout1[:, 0, :], 1.0 / N)
            nc.vector.tensor_mul(mm1[:], mean1[:], mean1[:])
            nc.vector.scalar_tensor_tensor(
                var1[:], arout1[:, 1, :], 1.0 / N, mm1[:],
                op0=ALU.mult, op1=ALU.subtract,
            )
            nc.vector.tensor_scalar_add(var1[:], var1[:], EPS)
            nc.scalar.activation(rstd1[:], var1[:], AF.Sqrt)
            nc.vector.reciprocal(rstd1[:], rstd1[:])
            # one Newton step for rsqrt accuracy: r = r*(1.5 - 0.5*v*r^2)
            nc.vector.tensor_mul(mm1[:], rstd1[:], rstd1[:])
            nc.vector.tensor_mul(mm1[:], mm1[:], var1[:])
            nc.vector.tensor_scalar(
                mm1[:], mm1[:], -0.5, 1.5, op0=ALU.mult, op1=ALU.add,
            )
            nc.vector.tensor_mul(rstd1[:], rstd1[:], mm1[:])
            nc.vector.tensor_mul(scale1[:], gb1_sb[:, 0, :], rstd1[:])
            nc.vector.tensor_mul(mm1[:], scale1[:], mean1[:])
            nc.vector.tensor_sub(shift1[:], gb1_sb[:, 1, :], mm1[:])

            # ---- normalize + GELU (in place, bf16), then matmul into PSUM ----
            for c in range(C):
                for q in range(PIECES):
                    nc.scalar.activation(
                        h1[q][:, c, :], h1[q][:, c, :], AF.Gelu,
                        bias=shift1[:, c:c + 1], scale=scale1[:, c:c + 1],
                    )

            ps_out = [ps.tile([128, NT], f32, tag=f"po{e}", name=f"po{e}") for e in range(CE)]
            arin2 = sb.tile([128, 2, CE], f32, tag="arin2", name="arin2")
            trash2 = sb.tile([128, NT], bf16, tag="trash2", name="trash2")
            for e in range(CE):
                for c in range(C):
                    lhsT = w2t_sb[:, c, e * 128:(e + 1) * 128]
                    for q in range(PIECES):
                        nc.tensor.matmul(
                            ps_out[e][:, q * PT:(q + 1) * PT],
                            lhsT,
                            h1[q][:, c, :],
                            start=(c == 0),
                            stop=(c == C - 1),
                        )
                # BN2 partial stats for this emb chunk overlap the next chunk's matmuls
                nc.vector.reduce_sum(arin2[:, 0, e:e + 1], ps_out[e][:], axis=AX.X)
                nc.scalar.activation(
                    trash2[:], ps_out[e][:], AF.Square,
                    accum_out=arin2[:, 1, e:e + 1],
                )
            ar2_in = dram.tile([128, 2, CE], f32, tag="ar2i", name="ar2i")
            ar2_out = dram.tile([128, 2, CE], f32, tag="ar2o", name="ar2o")
            nc.sync.dma_start(ar2_in[:], arin2[:])
            nc.gpsimd.collective_compute(
                "AllReduce", ALU.add, replica_groups=RG,
                ins=[ar2_in.opt()], outs=[ar2_out.opt()],
            )
            arout2 = sb.tile([128, 2, CE], f32, tag="arout2", name="arout2")
            nc.sync.dma_start(arout2[:], ar2_out[:])

            mean2 = sb.tile([128, CE], f32, tag="mean2", name="mean2")
            var2 = sb.tile([128, CE], f32, tag="var2", name="var2")
            mm2 = sb.tile([128, CE], f32, tag="mm2", name="mm2")
            rstd2 = sb.tile([128, CE], f32, tag="rstd2", name="rstd2")
            scale2 = sb.tile([128, CE], f32, tag="scale2", name="scale2")
            shift2 = sb.tile([128, CE], f32, tag="shift2", name="shift2")
            nc.vector.tensor_scalar_mul(mean2[:], arout2[:, 0, :], 1.0 / N)
            nc.vector.tensor_mul(mm2[:], mean2[:], mean2[:])
            nc.vector.scalar_tensor_tensor(
                var2[:], arout2[:, 1, :], 1.0 / N, mm2[:],
                op0=ALU.mult, op1=ALU.subtract,
            )
            nc.vector.tensor_scalar_add(var2[:], var2[:], EPS)
            nc.scalar.activation(rstd2[:], var2[:], AF.Sqrt)
            nc.vector.reciprocal(rstd2[:], rstd2[:])
            nc.vector.tensor_mul(mm2[:], rstd2[:], rstd2[:])
            nc.vector.tensor_mul(mm2[:], mm2[:], var2[:])
            nc.vector.tensor_scalar(
                mm2[:], mm2[:], -0.5, 1.5, op0=ALU.mult, op1=ALU.add,
            )
            nc.vector.tensor_mul(rstd2[:], rstd2[:], mm2[:])
            nc.vector.tensor_mul(scale2[:], gb2_sb[:, 0, :], rstd2[:])
            nc.vector.tensor_mul(mm2[:], scale2[:], mean2[:])
            nc.vector.tensor_sub(shift2[:], gb2_sb[:, 1, :], mm2[:])

            # ---- final normalize + GELU, write emb-major output ----
            out_sb = sb.tile([128, CE, NT], f32, tag="out", name="out")
            for e in range(CE):
                nc.scalar.activation(
                    out_sb[:, e, :], ps_out[e][:], AF.Gelu,
                    bias=shift2[:, e:e + 1], scale=scale2[:, e:e + 1],
                )
            nc.sync.dma_start(out[:], out_sb[:])

    nc.compile()
    _CACHE["nc"] = nc
    return nc


def kernel(x_t, W1, b1, g1, beta1, W2, b2, g2, beta2):
    from concourse.bass_utils import run_bass_kernel_spmd

    nc = _build_program()

    # b1/b2 cancel inside the BatchNorms (mean subtraction), so they are unused.
    table = np.ascontiguousarray(np.asarray(W1, dtype=np.float32).T).astype(BF16)
    w2t = np.ascontiguousarray(
        np.asarray(W2, dtype=np.float32).reshape(EMB, C, 128).transpose(2, 1, 0)
    ).astype(BF16)
    g1r = np.asarray(g1, dtype=np.float32).reshape(C, 128).T
    b1r = np.asarray(beta1, dtype=np.float32).reshape(C, 128).T
    gb1 = np.ascontiguousarray(np.stack([g1r, b1r], axis=1))      # [128, 2, C]
    g2r = np.asarray(g2, dtype=np.float32).reshape(CE, 128).T
    b2r = np.asarray(beta2, dtype=np.float32).reshape(CE, 128).T
    gb2 = np.ascontiguousarray(np.stack([g2r, b2r], axis=1))      # [128, 2, CE]

    x = np.asarray(x_t).astype(np.int64)
    in_maps = []
    for i in range(NCORES):
        xl = x[i * NT:(i + 1) * NT].astype(np.int16)
        wrapped = xl.reshape(NT // 16, 16).T                      # [16, NT//16]
        idx = np.ascontiguousarray(np.tile(wrapped, (8, 1)))      # [128, NT//16]
        in_maps.append(
            {"table": table, "idx": idx, "w2t": w2t, "gb1": gb1, "gb2": gb2}
        )

    _CACHE["in_maps"] = in_maps
    res = run_bass_kernel_spmd(nc, in_maps, list(range(NCORES)))

    shards = []
    for i in range(NCORES):
        o = res.results[i]["out"]                                 # [128, CE, NT]
        shards.append(o.transpose(2, 1, 0).reshape(NT, EMB))      # [NT, EMB]
    return np.ascontiguousarray(np.concatenate(shards, axis=0)).astype(np.float32)
